# revision 16
# baseline (speedup 1.0000x reference)
"""CRF forward-score kernel for Trainium2 (8 NeuronCores, data-parallel over batch).

Math: reference computes mean_b(forward_score(b) - gold_score(b)).

forward_score via the forward algorithm = a sequential log-semiring scan:
    alpha_t[j] = logsumexp_i(alpha_{t-1}[i] + trans[i,j]) + feat_t[j]
In exp-domain with E = exp(trans), F_t = exp(feat_t - c):
    P_t = (E^T P_{t-1}) * F_t          (P in [tag, batch] layout, per core batch=64)

The 512-step serial chain is halved by running the forward scan (t=0..255) and
an independent backward scan r_t = (E r_{t+1}) * F_t (t=511..256) concurrently,
joining in the middle:  score = log sum_i P_255[i] * (E R_256)[i] + 512*c + corr.

Both scans are packed into ONE [128, 64] state (fwd rows 0:64, bwd rows 64:128),
so each macro step i is a single stationary-blockdiag matmul
    psum = [[E, 0], [0, E^T]]^T_applied @ state        (PE)
and a single elementwise multiply                       (DVE)
    state' = psum * FTcol(i+1)
where FTcol(c) [128, 64] holds exp(feat_c - c_shift) transposed for the fwd half
and exp(feat_{512-c} - c_shift) for the bwd half.  FT columns are produced by PE
transposes (identity matmuls) into PSUM and exponentiated in bulk on ACT —
the DMA transpose xbar is far too slow for this (measured ~1.2us/tile).

Renormalization: constant shift c (mean log-growth, host-estimated) folded into
F; residual per-batch drift removed every 32 macros by scaling one FT column by
1/colsum(state-half) (masked-ones matmul -> reciprocal -> rank-1 broadcast
matmul -> fold multiply), accumulating log(colsum) for the score.
"""

import numpy as np
import ml_dtypes

B, S, T = 512, 512, 64
NCORES = 8
BC = B // NCORES  # 64 batch per core
HALF = S // 2  # 256 macro steps
RENORM_EVERY = 32
RENORM_LAG = 6  # snapshot this many macros before the fold
STG = 16  # timesteps per staging tile
FTC = 4  # FT columns per FT tile

C_SHIFT = 5.17  # overwritten at kernel() time before _build


def _patch_tile_drain():
    """This walrus build rejects >1 sync wait per instruction.  Split excess
    waits onto preceding same-engine drains at lowering commit time, and fix
    the multi-wait tail drain the same way."""
    import concourse.mybir as mybir
    import concourse.tile as tile_mod

    if getattr(tile_mod.TileContext, "_drain_patched", False):
        return

    def _drain_and_barrier(self, tick_clock, wait_clock):
        nc = self.nc
        drain_inst = nc.sync.drain()
        wait_clock.add_sem_waits(
            drain_inst.ins, tile_mod.ScopedClock({None: tick_clock.global_clock})
        )
        si = drain_inst.ins.sync_info
        if si is not None and si.on_wait is not None and len(si.on_wait) > 1:
            waits = list(si.on_wait)
            si.on_wait = waits[:1]
            for w in waits[1:]:
                nop_inst = nc.sync.nop(nofuse=True, hint="drain_wait_spill")
                nsi = nop_inst.ins.sync_info
                if nsi is None:
                    nop_inst.ins.sync_info = mybir.SyncInfo(on_wait=[w], on_update=[])
                else:
                    nsi.on_wait = [w]
        nc.all_engine_barrier()
        assert self.sems is not None
        popped = nc._tile_sem_poison_stack.pop()
        assert popped is self._sem_poison
        nc.clear_and_free_semaphores(list(self.sems.allocated().values()))
        nc.all_engine_barrier()

    tile_mod.TileContext._drain_and_barrier = _drain_and_barrier

    _orig_commit = tile_mod.TileContext._commit_instruction

    def _commit_split(self, inst, lazy_reg_writes=True):
        si = getattr(inst, "sync_info", None)
        if si is not None and si.on_wait is not None and len(si.on_wait) > 1:
            waits = list(si.on_wait)
            si.on_wait = [waits[0]]
            for w in waits[1:]:
                nop_inst = self.nc.engines[inst.engine].drain(fusable=False)
                nsi = nop_inst.ins.sync_info
                if nsi is None:
                    nop_inst.ins.sync_info = mybir.SyncInfo(on_wait=[w], on_update=[])
                else:
                    nsi.on_wait = [w]
        return _orig_commit(self, inst, lazy_reg_writes)

    tile_mod.TileContext._commit_instruction = _commit_split
    tile_mod.TileContext._drain_patched = True


def _build():
    import concourse.bass as bass
    import concourse.mybir as mybir
    from concourse.tile import TileContext

    _patch_tile_drain()
    dt = mybir.dt

    nc = bass.Bass("TRN2", target_bir_lowering=False, debug=False, num_devices=1)
    feats_d = nc.dram_tensor("feats", [BC, S, T], dt.bfloat16, kind="ExternalInput")
    bd_d = nc.dram_tensor("BD", [2 * T, 2 * T], dt.bfloat16, kind="ExternalInput")
    id_d = nc.dram_tensor("IDN", [T, T], dt.bfloat16, kind="ExternalInput")
    out_d = nc.dram_tensor("out", [1, 3 * T], dt.float32, kind="ExternalOutput")

    with TileContext(nc) as tc:
        with (
            tc.tile_pool(name="const", bufs=1) as constp,
            tc.tile_pool(name="stgf", bufs=3) as stgfp,
            tc.tile_pool(name="stgb", bufs=3) as stgbp,
            tc.tile_pool(name="ftp", bufs=4) as ftp,
            tc.tile_pool(name="state", bufs=8) as statep,
            tc.tile_pool(name="ftmod", bufs=2) as ftmodp,
            tc.tile_pool(name="small", bufs=4) as smallp,
            tc.tile_pool(name="ps", bufs=2, space="PSUM") as psp,
            tc.tile_pool(name="pt", bufs=2, space="PSUM") as ptp,
            tc.tile_pool(name="pmisc", bufs=1, space="PSUM") as pmiscp,
        ):
            # ---- constants ----
            bd_sb = constp.tile([2 * T, 2 * T], dt.bfloat16, tag="bd")
            ident = constp.tile([T, T], dt.bfloat16, tag="ident")
            onesF = constp.tile([2 * T, 1], dt.bfloat16, tag="onesF")
            onesB = constp.tile([2 * T, 1], dt.bfloat16, tag="onesB")
            ones_col = constp.tile([T, 1], dt.bfloat16, tag="ones_col")
            ones_row = constp.tile([1, T], dt.float32, tag="ones_row")
            acc = constp.tile([1, 2 * T], dt.float32, tag="acc")
            cbias = constp.tile([2 * T, 1], dt.float32, tag="cbias")
            nc.sync.dma_start(out=bd_sb[:], in_=bd_d[:])
            nc.sync.dma_start(out=ident[:], in_=id_d[:])
            nc.gpsimd.memset(onesF[:T], 1.0)
            nc.gpsimd.memset(onesF[T:], 0.0)
            nc.gpsimd.memset(onesB[:T], 0.0)
            nc.gpsimd.memset(onesB[T:], 1.0)
            nc.gpsimd.memset(ones_col[:], 1.0)
            nc.gpsimd.memset(ones_row[:], 1.0)
            nc.gpsimd.memset(acc[:], 0.0)
            nc.gpsimd.memset(cbias[:], -C_SHIFT)

            # ---- feats staging (plain DMA, natural layout) ----
            # fwd stage tile k covers t in [16k, 16k+16); bwd tile k covers
            # t in [512-16(k+1), 512-16k) i.e. descending from the top.
            stgf_tiles, stgb_tiles = {}, {}

            def stage_fwd(k):
                st = stgfp.tile([BC, STG * T], dt.bfloat16)
                nc.sync.dma_start(
                    out=st[:],
                    in_=feats_d[:, STG * k : STG * (k + 1), :].rearrange(
                        "b t j -> b (t j)"
                    ),
                )
                stgf_tiles[k] = st

            def stage_bwd(k):
                st = stgbp.tile([BC, STG * T], dt.bfloat16)
                nc.sync.dma_start(
                    out=st[:],
                    in_=feats_d[:, S - STG * (k + 1) : S - STG * k, :].rearrange(
                        "b t j -> b (t j)"
                    ),
                )
                stgb_tiles[k] = st

            def fwd_nat(t):  # natural-layout [BC, T] slice of timestep t
                k = t // STG
                o = t % STG
                return stgf_tiles[k][:, o * T : (o + 1) * T]

            def bwd_nat(t):
                k = (S - 1 - t) // STG
                o = t - (S - STG * (k + 1))
                return stgb_tiles[k][:, o * T : (o + 1) * T]

            # ---- FT tiles ----
            # FT column c [128, 64]: rows 0:64 = exp(feats[:, c, :].T - cs)
            #                        rows 64:128 = exp(feats[:, 512-c, :].T - cs)
            # tile m covers columns [4m, 4m+4); macro i consumes column i+1.
            ft_tiles = {}

            def make_ft(m):
                pt = ptp.tile([2 * T, FTC * T], dt.bfloat16)
                for lc in range(FTC):
                    c = FTC * m + lc
                    if c <= 256:
                        nc.tensor.transpose(
                            pt[:T, lc * T : (lc + 1) * T], fwd_nat(c), ident[:]
                        )
                    if 1 <= c <= 256:
                        nc.tensor.transpose(
                            pt[T:, lc * T : (lc + 1) * T], bwd_nat(S - c), ident[:]
                        )
                    elif c == 0:
                        nc.tensor.transpose(
                            pt[T:, lc * T : (lc + 1) * T], fwd_nat(0), ident[:]
                        )
                    else:  # c > 256: pad with anything valid
                        nc.tensor.transpose(
                            pt[:T, lc * T : (lc + 1) * T], fwd_nat(256), ident[:]
                        )
                        nc.tensor.transpose(
                            pt[T:, lc * T : (lc + 1) * T], fwd_nat(256), ident[:]
                        )
                ft = ftp.tile([2 * T, FTC * T], dt.bfloat16)
                nc.scalar.activation(
                    ft[:], pt[:], mybir.ActivationFunctionType.Exp, bias=cbias[:]
                )
                ft_tiles[m] = ft

            def ft_col(c):
                m = c // FTC
                lc = c % FTC
                return ft_tiles[m][:, lc * T : (lc + 1) * T]

            # prime staging + first FT tiles
            stage_fwd(0)
            stage_bwd(0)
            for m in range(2):
                make_ft(m)

            # ---- initial state: rows 0:64 = FT(0) fwd, rows 64:128 = FT(511) ----
            s0 = statep.tile([2 * T, BC], dt.bfloat16, tag="s")
            nc.scalar.copy(s0[:T], ft_col(0)[:T])
            nc.scalar.copy(s0[T:], ft_col(1)[T:])
            state = s0
            prev_state = None  # state from the previous macro (P_255 lives here)

            renorm_snap = None

            for i in range(HALF + 1):
                # prefetch staging and FT tiles
                if i % STG == 4:
                    kf = (i + STG) // STG
                    if kf * STG <= 256 and kf not in stgf_tiles:
                        stage_fwd(kf)
                    if kf not in stgb_tiles and S - STG * (kf + 1) >= 255:
                        stage_bwd(kf)
                if i % FTC == 0:
                    m = (i + FTC + 1) // FTC
                    if m * FTC <= 257 and m not in ft_tiles:
                        make_ft(m)

                # ---- renorm bookkeeping (off the critical chain) ----
                ri = i % RENORM_EVERY
                if ri == RENORM_EVERY - 1 - RENORM_LAG and i < HALF - 8:
                    renorm_snap = state
                fold_now = (
                    ri == RENORM_EVERY - 1 and i < HALF - 2 and renorm_snap is not None
                )
                ft_in = ft_col(i + 1) if i < HALF else None
                if fold_now:
                    scol = pmiscp.tile([1, 2 * T], dt.float32, tag="scol")
                    nc.tensor.matmul(
                        scol[:, :T], onesF[:], renorm_snap[:], start=True, stop=True
                    )
                    nc.tensor.matmul(
                        scol[:, T:], onesB[:], renorm_snap[:], start=True, stop=True
                    )
                    inv = smallp.tile([1, 2 * T], dt.float32, tag="inv")
                    nc.vector.reciprocal(inv[:], scol[:])
                    lns = smallp.tile([1, 2 * T], dt.float32, tag="lns")
                    nc.scalar.activation(
                        lns[:], scol[:], mybir.ActivationFunctionType.Ln
                    )
                    nc.vector.tensor_add(acc[:], acc[:], lns[:])
                    invbc = pmiscp.tile([2 * T, BC], dt.float32, tag="invbc")
                    nc.tensor.matmul(
                        invbc[:T], ones_row[:], inv[:, :T], start=True, stop=True
                    )
                    nc.tensor.matmul(
                        invbc[T:], ones_row[:], inv[:, T:], start=True, stop=True
                    )
                    ftm = ftmodp.tile([2 * T, BC], dt.bfloat16, tag="ftm")
                    nc.vector.tensor_mul(ftm[:], ft_in, invbc[:])
                    ft_in = ftm[:]

                # ---- chain step ----
                ps = psp.tile([2 * T, BC], dt.float32, tag="ps")
                nc.tensor.matmul(ps[:], bd_sb[:], state[:], start=True, stop=True)
                if i < HALF:
                    new_s = statep.tile([2 * T, BC], dt.bfloat16, tag="s")
                    nc.vector.tensor_mul(new_s[:], ps[:], ft_in)
                    prev_state = state
                    state = new_s

                # release consumed staging tiles
                done_f = (i + 1) // STG - 1
                if (i + 1) % STG == 0 and done_f in stgf_tiles and done_f >= 0:
                    pass  # dict retention is fine; pool rotation handles reuse

            # Tail: ps rows 64:128 = E @ R_256 = B*;  P_255 = prev_state rows 0:64
            # (state after macro 254; at i=255 'state' advanced once more).
            bstar = smallp.tile([2 * T, BC], dt.float32, tag="bstar")
            nc.scalar.copy(bstar[T:], ps[T:])
            bstar0 = smallp.tile([T, BC], dt.float32, tag="bstar0")
            nc.sync.dma_start(out=bstar0[:], in_=bstar[T:])
            v = smallp.tile([T, BC], dt.bfloat16, tag="v")
            nc.vector.tensor_mul(v[:], bstar0[:], prev_state[:T])
            dot = pmiscp.tile([1, T], dt.float32, tag="dot")
            nc.tensor.matmul(dot[:], ones_col[:], v[:], start=True, stop=True)
            lnd = smallp.tile([1, T], dt.float32, tag="lnd")
            nc.scalar.activation(lnd[:], dot[:], mybir.ActivationFunctionType.Ln)
            nc.sync.dma_start(out=out_d[:, : 2 * T], in_=acc[:])
            nc.sync.dma_start(out=out_d[:, 2 * T :], in_=lnd[:])

    return nc


def _estimate_c(feats, transitions):
    """Mean per-step log-growth of max_j alpha_t[j], from a small batch sample."""
    nb = 8
    a = feats[:nb, 0].astype(np.float64)
    etr = np.exp(transitions.astype(np.float64))
    m0 = a.max(axis=1).mean()
    for t in range(1, S):
        m = a.max(axis=1, keepdims=True)
        a = np.log(np.exp(a - m) @ etr) + m + feats[:nb, t]
    return float((a.max(axis=1).mean() - m0) / (S - 1))


LAST_EXEC_NS = None
LAST_TRACE = None


def kernel(feats, tags, transitions, _trace=False):
    global C_SHIFT, LAST_EXEC_NS, LAST_TRACE
    feats = np.asarray(feats, dtype=np.float32)
    tags = np.asarray(tags)
    transitions = np.asarray(transitions, dtype=np.float32)

    C_SHIFT = float(_estimate_c(feats, transitions))

    from concourse.bass_utils import run_bass_kernel_spmd

    nc = _build()

    e = np.exp(transitions.astype(np.float64))
    bd = np.zeros((2 * T, 2 * T), dtype=np.float64)
    bd[:T, :T] = e  # fwd: out = E^T P
    bd[T:, T:] = e.T  # bwd: out = E R
    bd = bd.astype(ml_dtypes.bfloat16)
    idn = np.eye(T, dtype=ml_dtypes.bfloat16)
    feats_bf = feats.astype(ml_dtypes.bfloat16)
    in_maps = [
        {"feats": feats_bf[ci * BC : (ci + 1) * BC], "BD": bd, "IDN": idn}
        for ci in range(NCORES)
    ]
    res = run_bass_kernel_spmd(nc, in_maps, list(range(NCORES)), trace=_trace)
    LAST_EXEC_NS = res.exec_time_ns
    LAST_TRACE = res.profile_json

    scores = np.zeros(B)
    for ci in range(NCORES):
        o = res.results[ci]["out"].reshape(3 * T).astype(np.float64)
        scores[ci * BC : (ci + 1) * BC] = o[:T] + o[T : 2 * T] + o[2 * T :]
    fwd = scores + S * C_SHIFT

    # gold path score (host: trivial gather arithmetic)
    tags_i = tags.astype(np.int64)
    emit = np.take_along_axis(feats, tags_i[:, :, None], axis=2)[..., 0].sum(axis=1)
    trans = transitions[tags_i[:, :-1], tags_i[:, 1:]].sum(axis=1)
    gold = emit.astype(np.float64) + trans.astype(np.float64)

    return np.float32(np.mean(fwd - gold))


# revision 28
# speedup vs baseline: 1.4102x; 1.4102x over previous
"""CRF forward-score kernel for Trainium2 (8 NeuronCores, data-parallel over batch).

Math: reference computes mean_b(forward_score(b) - gold_score(b)).

forward_score via the forward algorithm = a sequential log-semiring scan:
    alpha_t[j] = logsumexp_i(alpha_{t-1}[i] + trans[i,j]) + feat_t[j]
In exp-domain with E = exp(trans), F_t = exp(feat_t - c):
    P_t = (E^T P_{t-1}) * F_t          (P in [tag, batch] layout, per core batch=64)

The 512-step serial chain is halved by running the forward scan (t=0..255) and
an independent backward scan r_t = (E r_{t+1}) * F_t (t=511..256) concurrently,
joining in the middle:  score = log sum_i P_255[i] * (E R_256)[i] + 512*c + corr.

Both scans are packed into ONE [128, 64] state (fwd rows 0:64, bwd rows 64:128),
so each macro step i is a single stationary-blockdiag matmul
    psum = [[E, 0], [0, E^T]]^T_applied @ state        (PE)
and a single elementwise multiply                       (DVE)
    state' = psum * FTcol(i+1)
where FTcol(c) [128, 64] holds exp(feat_c - c_shift) transposed for the fwd half
and exp(feat_{512-c} - c_shift) for the bwd half.  FT columns are produced by PE
transposes (identity matmuls) into PSUM and exponentiated in bulk on ACT —
the DMA transpose xbar is far too slow for this (measured ~1.2us/tile).

Renormalization: constant shift c (mean log-growth, host-estimated) folded into
F; residual per-batch drift removed every 32 macros by scaling one FT column by
1/colsum(state-half) (masked-ones matmul -> reciprocal -> rank-1 broadcast
matmul -> fold multiply), accumulating log(colsum) for the score.
"""

import numpy as np
import ml_dtypes

B, S, T = 512, 512, 64
NCORES = 8
BC = B // NCORES  # 64 batch per core
HALF = S // 2  # 256 macro steps
RENORM_EVERY = 32
RENORM_LAG = 6  # snapshot this many macros before the fold
STG = 16  # timesteps per staging tile
FTC = 4  # FT columns per FT tile

C_SHIFT = 5.17  # overwritten at kernel() time before _build


def _patch_tile_drain():
    """This walrus build rejects >1 sync wait per instruction.  Split excess
    waits onto preceding same-engine drains at lowering commit time, and fix
    the multi-wait tail drain the same way."""
    import concourse.mybir as mybir
    import concourse.tile as tile_mod

    if getattr(tile_mod.TileContext, "_drain_patched", False):
        return

    def _drain_and_barrier(self, tick_clock, wait_clock):
        nc = self.nc
        drain_inst = nc.sync.drain()
        wait_clock.add_sem_waits(
            drain_inst.ins, tile_mod.ScopedClock({None: tick_clock.global_clock})
        )
        si = drain_inst.ins.sync_info
        if si is not None and si.on_wait is not None and len(si.on_wait) > 1:
            waits = list(si.on_wait)
            si.on_wait = waits[:1]
            for w in waits[1:]:
                nop_inst = nc.sync.nop(nofuse=True, hint="drain_wait_spill")
                nsi = nop_inst.ins.sync_info
                if nsi is None:
                    nop_inst.ins.sync_info = mybir.SyncInfo(on_wait=[w], on_update=[])
                else:
                    nsi.on_wait = [w]
        nc.all_engine_barrier()
        assert self.sems is not None
        popped = nc._tile_sem_poison_stack.pop()
        assert popped is self._sem_poison
        nc.clear_and_free_semaphores(list(self.sems.allocated().values()))
        nc.all_engine_barrier()

    tile_mod.TileContext._drain_and_barrier = _drain_and_barrier

    _orig_commit = tile_mod.TileContext._commit_instruction

    def _commit_split(self, inst, lazy_reg_writes=True):
        si = getattr(inst, "sync_info", None)
        if si is not None and si.on_wait is not None and len(si.on_wait) > 1:
            waits = list(si.on_wait)
            si.on_wait = [waits[0]]
            for w in waits[1:]:
                nop_inst = self.nc.engines[inst.engine].drain(fusable=False)
                nsi = nop_inst.ins.sync_info
                if nsi is None:
                    nop_inst.ins.sync_info = mybir.SyncInfo(on_wait=[w], on_update=[])
                else:
                    nsi.on_wait = [w]
        return _orig_commit(self, inst, lazy_reg_writes)

    tile_mod.TileContext._commit_instruction = _commit_split
    tile_mod.TileContext._drain_patched = True


def _build():
    import concourse.bass as bass
    import concourse.mybir as mybir
    from concourse.tile import TileContext

    _patch_tile_drain()
    dt = mybir.dt

    nc = bass.Bass("TRN2", target_bir_lowering=False, debug=False, num_devices=1)
    # FI[b, c, 0:64] = feats[b, c, :], FI[b, c, 64:128] = feats[b, 512-c, :]
    # (host-interleaved so one PE transpose yields a stacked fwd/bwd FT column)
    feats_d = nc.dram_tensor(
        "FI", [BC, HALF + 1, 2 * T], dt.bfloat16, kind="ExternalInput"
    )
    bd_d = nc.dram_tensor("BD", [2 * T, 2 * T], dt.bfloat16, kind="ExternalInput")
    id_d = nc.dram_tensor("IDN", [T, T], dt.bfloat16, kind="ExternalInput")
    out_d = nc.dram_tensor("out", [1, 3 * T], dt.float32, kind="ExternalOutput")

    with TileContext(nc) as tc:
        with (
            tc.tile_pool(name="const", bufs=1) as constp,
            tc.tile_pool(name="ftp", bufs=4) as ftp,
            tc.tile_pool(name="state", bufs=8) as statep,
            tc.tile_pool(name="ftmod", bufs=2) as ftmodp,
            tc.tile_pool(name="small", bufs=4) as smallp,
            tc.tile_pool(name="ps", bufs=2, space="PSUM") as psp,
            tc.tile_pool(name="pt", bufs=2, space="PSUM") as ptp,
            tc.tile_pool(name="pmisc", bufs=1, space="PSUM") as pmiscp,
        ):
            # ---- constants ----
            bd_sb = constp.tile([2 * T, 2 * T], dt.bfloat16, tag="bd")
            ident = constp.tile([T, T], dt.bfloat16, tag="ident")
            onesF = constp.tile([2 * T, 1], dt.bfloat16, tag="onesF")
            onesB = constp.tile([2 * T, 1], dt.bfloat16, tag="onesB")
            ones_col = constp.tile([T, 1], dt.bfloat16, tag="ones_col")
            ones_row = constp.tile([1, T], dt.float32, tag="ones_row")
            acc = constp.tile([1, 2 * T], dt.float32, tag="acc")
            cbias = constp.tile([2 * T, 1], dt.float32, tag="cbias")
            nc.sync.dma_start(out=bd_sb[:], in_=bd_d[:])
            nc.sync.dma_start(out=ident[:], in_=id_d[:])
            nc.gpsimd.memset(onesF[:T], 1.0)
            nc.gpsimd.memset(onesF[T:], 0.0)
            nc.gpsimd.memset(onesB[:T], 0.0)
            nc.gpsimd.memset(onesB[T:], 1.0)
            nc.gpsimd.memset(ones_col[:], 1.0)
            nc.gpsimd.memset(ones_row[:], 1.0)
            nc.gpsimd.memset(acc[:], 0.0)
            nc.gpsimd.memset(cbias[:], -C_SHIFT)

            # ---- feats staging: host-interleaved shard resident in SBUF ----
            # STALL[b, c*128 + x] = FI[b, c, x]; 8 big DMAs in ascending c
            # order (the chain consumes c ascending, both directions at once).
            NC_COLS = HALF + 1
            stall = constp.tile([BC, NC_COLS * 2 * T], dt.bfloat16, tag="stall")
            nchunk = 16
            per = (NC_COLS + nchunk - 1) // nchunk  # columns per chunk
            for k in range(nchunk):
                c0, c1 = per * k, min(per * (k + 1), NC_COLS)
                if c0 >= c1:
                    continue
                nc.sync.dma_start(
                    out=stall[:, c0 * 2 * T : c1 * 2 * T],
                    in_=feats_d[:, c0:c1, :].rearrange("b c x -> b (c x)"),
                )

            # ---- FT tiles ----
            # FT column c [128, 64]: rows 0:64 = exp(feats[:, c, :].T - cs)
            #                        rows 64:128 = exp(feats[:, 512-c, :].T - cs)
            # One PE transpose per column (contiguous [64, 128] STALL slice).
            # Tile m covers columns [4m, 4m+4).
            ft_tiles = {}

            def make_ft(m):
                pt = ptp.tile([2 * T, FTC * T], dt.bfloat16)
                for lc in range(FTC):
                    c = min(FTC * m + lc, 256)
                    blk = stall[:, c * 2 * T : (c + 1) * 2 * T]
                    nc.tensor.transpose(
                        pt[:, lc * T : (lc + 1) * T], blk, ident[:]
                    )
                ft = ftp.tile([2 * T, FTC * T], dt.bfloat16)
                nc.scalar.activation(
                    ft[:], pt[:], mybir.ActivationFunctionType.Exp, bias=cbias[:]
                )
                ft_tiles[m] = ft

            def ft_col(c):
                m = c // FTC
                lc = c % FTC
                return ft_tiles[m][:, lc * T : (lc + 1) * T]

            for m in range(2):
                make_ft(m)

            # ---- initial state: rows 0:64 = FT(0) fwd, rows 64:128 = FT(511) ----
            s0 = statep.tile([2 * T, BC], dt.bfloat16, tag="s")
            nc.scalar.copy(s0[:T], ft_col(0)[:T])
            nc.scalar.copy(s0[T:], ft_col(1)[T:])
            state = s0
            prev_state = None  # state from the previous macro (P_255 lives here)

            renorm_snap = None

            for i in range(HALF + 1):
                # prefetch FT tiles
                if i % FTC == 0:
                    m = (i + FTC + 1) // FTC
                    if m * FTC <= 257 and m not in ft_tiles:
                        make_ft(m)

                # ---- renorm bookkeeping (off the critical chain) ----
                ri = i % RENORM_EVERY
                if ri == RENORM_EVERY - 1 - RENORM_LAG and i < HALF - 8:
                    renorm_snap = state
                fold_now = (
                    ri == RENORM_EVERY - 1 and i < HALF - 2 and renorm_snap is not None
                )
                ft_in = ft_col(i + 1) if i < HALF else None
                if fold_now:
                    scol = pmiscp.tile([1, 2 * T], dt.float32, tag="scol")
                    nc.tensor.matmul(
                        scol[:, :T], onesF[:], renorm_snap[:], start=True, stop=True
                    )
                    nc.tensor.matmul(
                        scol[:, T:], onesB[:], renorm_snap[:], start=True, stop=True
                    )
                    scol_sb = smallp.tile([1, 2 * T], dt.float32, tag="scol_sb")
                    nc.scalar.copy(scol_sb[:], scol[:])
                    inv = smallp.tile([1, 2 * T], dt.float32, tag="inv")
                    nc.vector.reciprocal(inv[:], scol_sb[:])
                    # account for exactly the factor applied: acc -= ln(inv)
                    lns = smallp.tile([1, 2 * T], dt.float32, tag="lns")
                    nc.scalar.activation(
                        lns[:], inv[:], mybir.ActivationFunctionType.Ln
                    )
                    nc.vector.tensor_sub(acc[:], acc[:], lns[:])
                    invbc = pmiscp.tile([2 * T, BC], dt.float32, tag="invbc")
                    nc.tensor.matmul(
                        invbc[:T], ones_row[:], inv[:, :T], start=True, stop=True
                    )
                    nc.tensor.matmul(
                        invbc[T:], ones_row[:], inv[:, T:], start=True, stop=True
                    )
                    ftm = ftmodp.tile([2 * T, BC], dt.bfloat16, tag="ftm")
                    nc.vector.tensor_mul(ftm[:], ft_in, invbc[:])
                    ft_in = ftm[:]

                # ---- chain step ----
                ps = psp.tile([2 * T, BC], dt.float32, tag="ps")
                nc.tensor.matmul(ps[:], bd_sb[:], state[:], start=True, stop=True)
                if i < HALF:
                    new_s = statep.tile([2 * T, BC], dt.bfloat16, tag="s")
                    nc.vector.tensor_mul(new_s[:], ps[:], ft_in)
                    prev_state = state
                    state = new_s


            # Tail: ps rows 64:128 = E @ R_256 = B*;  P_255 = prev_state rows 0:64
            # (state after macro 254; at i=255 'state' advanced once more).
            bstar = smallp.tile([2 * T, BC], dt.float32, tag="bstar")
            nc.scalar.copy(bstar[T:], ps[T:])
            bstar0 = smallp.tile([T, BC], dt.float32, tag="bstar0")
            nc.sync.dma_start(out=bstar0[:], in_=bstar[T:])
            v = smallp.tile([T, BC], dt.bfloat16, tag="v")
            nc.vector.tensor_mul(v[:], bstar0[:], prev_state[:T])
            dot = pmiscp.tile([1, T], dt.float32, tag="dot")
            nc.tensor.matmul(dot[:], ones_col[:], v[:], start=True, stop=True)
            lnd = smallp.tile([1, T], dt.float32, tag="lnd")
            nc.scalar.activation(lnd[:], dot[:], mybir.ActivationFunctionType.Ln)
            nc.sync.dma_start(out=out_d[:, : 2 * T], in_=acc[:])
            nc.sync.dma_start(out=out_d[:, 2 * T :], in_=lnd[:])

    return nc


def _estimate_c(feats, transitions):
    """Mean per-step log-growth of max_j alpha_t[j], from a small batch sample."""
    nb = 8
    a = feats[:nb, 0].astype(np.float64)
    etr = np.exp(transitions.astype(np.float64))
    m0 = a.max(axis=1).mean()
    for t in range(1, S):
        m = a.max(axis=1, keepdims=True)
        a = np.log(np.exp(a - m) @ etr) + m + feats[:nb, t]
    return float((a.max(axis=1).mean() - m0) / (S - 1))


LAST_EXEC_NS = None
LAST_TRACE = None


def kernel(feats, tags, transitions, _trace=False):
    global C_SHIFT, LAST_EXEC_NS, LAST_TRACE
    feats = np.asarray(feats, dtype=np.float32)
    tags = np.asarray(tags)
    transitions = np.asarray(transitions, dtype=np.float32)

    C_SHIFT = float(_estimate_c(feats, transitions))

    from concourse.bass_utils import run_bass_kernel_spmd

    nc = _build()

    e = np.exp(transitions.astype(np.float64))
    bd = np.zeros((2 * T, 2 * T), dtype=np.float64)
    bd[:T, :T] = e  # fwd: out = E^T P
    bd[T:, T:] = e.T  # bwd: out = E R
    bd = bd.astype(ml_dtypes.bfloat16)
    idn = np.eye(T, dtype=ml_dtypes.bfloat16)
    feats_bf = feats.astype(ml_dtypes.bfloat16)
    fi = np.empty((B, HALF + 1, 2 * T), dtype=ml_dtypes.bfloat16)
    fi[:, :, :T] = feats_bf[:, : HALF + 1, :]
    fi[:, 1:, T:] = feats_bf[:, : HALF - 1 : -1, :]  # t = 511 down to 256
    fi[:, 0, T:] = feats_bf[:, 0, :]  # unused dummy
    in_maps = [
        {"FI": fi[ci * BC : (ci + 1) * BC], "BD": bd, "IDN": idn}
        for ci in range(NCORES)
    ]
    res = run_bass_kernel_spmd(nc, in_maps, list(range(NCORES)), trace=_trace)
    LAST_EXEC_NS = res.exec_time_ns
    LAST_TRACE = res.profile_json

    scores = np.zeros(B)
    for ci in range(NCORES):
        o = res.results[ci]["out"].reshape(3 * T).astype(np.float64)
        scores[ci * BC : (ci + 1) * BC] = o[:T] + o[T : 2 * T] + o[2 * T :]
    fwd = scores + S * C_SHIFT

    # gold path score (host: trivial gather arithmetic)
    tags_i = tags.astype(np.int64)
    emit = np.take_along_axis(feats, tags_i[:, :, None], axis=2)[..., 0].sum(axis=1)
    trans = transitions[tags_i[:, :-1], tags_i[:, 1:]].sum(axis=1)
    gold = emit.astype(np.float64) + trans.astype(np.float64)

    return np.float32(np.mean(fwd - gold))
